# revision 55
# baseline (speedup 1.0000x reference)
"""ALIF spiking-network forward + eligibility traces on 8 Trainium2 NeuronCores.

Data-parallel: batch 32 sharded 4 samples/core. The only sequential part is the
(B,H) spike recurrence (48 steps); the heavy per-sample eligibility-trace
matrices (eps/fe, H x I and H x H) are collapsed algebraically into per-(h,t)
scalars + a few small matmuls:

  eps_t = (rho - beta*psi_t) eps_{t-1} + psi_t (x) tr_t   (rank-1, row decay)
  fe_T  = sum_s W_s (x) tr_s,  W_s = psi_s (kap_s - beta R_s),
  R_s   = kap_s psi_s + c_{s+1} R_{s+1}                   (backward linear scan)
  tr    = La @ X  /  La'' @ Z                             (triangular matmul)

so  fe_in = W^T @ (La @ X),  fe_rec = W^T @ (La'' @ Z).
"""

import numpy as np

# ---------------------------------------------------------------- constants
ALPHA = float(np.exp(-1.0 / 20.0))
RHO = float(np.exp(-1.0 / 2000.0))
KAPPA = float(np.exp(-1.0 / 20.0))
BETA = 0.07
VTH = 0.6
GAMMA = 0.3

B, T, I, H, O = 32, 48, 128, 256, 64
NCORES = 8
BS = B // NCORES          # 4 samples per core
HT = H // 128             # 2 h-tiles
FW = HT * BS              # 8 = free width of loop state tiles
NB = 4                    # i_t PSUM bank rotation depth
TB = T // NB              # 12 time-cols per bank

C1_ = VTH * (1.0 - RHO)
W_ = 1.0 / ALPHA - 1.0 / RHO
U_ = C1_ / RHO
K1_ = W_ * (BETA + C1_) + U_ * (1.0 - RHO)
K2_ = W_ * C1_ + U_ * (1.0 - RHO)
M1_ = 1.0 / (ALPHA * W_)
N1_ = (VTH - U_ / W_) / ALPHA
Q0_ = VTH / ALPHA
B0_ = W_ * VTH + U_
# pre-update-B formulation (QOp reads B before AOp updates it)
UT_ = (BETA + VTH) / ALPHA
B0N_ = VTH + UT_
K1N_ = BETA + C1_ + UT_ * (1.0 - RHO)
K2N_ = C1_ + UT_ * (1.0 - RHO)
M2_ = RHO / ALPHA
M3_ = RHO / ALPHA - 1.0
N0_ = C1_ / ALPHA - (RHO / ALPHA - 1.0) * UT_

_OPS_REGISTERED = {}


def _register_dve_ops():
    """Two fused DVE ops for the loop state updates (idempotent)."""
    global _OPS_REGISTERED
    if _OPS_REGISTERED:
        return _OPS_REGISTERED
    import concourse.dve_ops as dve_ops
    from concourse.dve_ops import OPS, DveOp
    from concourse.dve_spec import Spec, Src0, Src1, C0, C1, C2, select, Zero, One, lower, relu, minn
    from concourse.dve_uop import DveOpSpec

    def mk(name, spec):
        for o in OPS:
            if o.name == name:
                return o
        from concourse.dve_ops import has_src1
        shas = {}
        for ver in ("v3", "v4"):
            s = DveOpSpec(name=name, opcode=0, uops=lower(spec, ver=ver), rd1_en=has_src1(spec))
            shas[ver] = s.sha(ver)
        op = DveOp(name, spec, subdim=False, uops_sha=shas)
        OPS.append(op)
        dve_ops.CUSTOM_DVE_SPECS[name] = spec
        dve_ops._SUB_OPCODE_FOR_NAME[name] = dve_ops._CUSTOM_DVE_ROW_BASE + len(OPS) - 1
        assert dve_ops._SUB_OPCODE_FOR_NAME[name] < 0x20
        return op

    # B' = rho*B + (d>0 ? k1 : k2)      (B = affine-transformed threshold state)
    aop = mk(
        "ALIF_BUPD",
        Spec(
            body=Src1 * C0 + select(Src0 > Zero, C1, C2),
            reference=lambda in0, in1, s0, s1, imm2: (
                np.asarray(in1) * s0 + np.where(np.asarray(in0) > 0, s1, imm2)
            ).astype(np.float32),
        ),
    )
    # q' = (d>0 ? m1*B' + n1 : B') - d   (q: next-step threshold, d_{t+1}=i-alpha*q)
    qop = mk(
        "ALIF_QUPD",
        Spec(
            body=select(Src0 > Zero, C0 * Src1 + C1, Src1) - Src0,
            reference=lambda in0, in1, s0, s1, imm2: (
                np.where(np.asarray(in0) > 0, s0 * np.asarray(in1) + s1, np.asarray(in1))
                - np.asarray(in0)
            ).astype(np.float32),
        ),
    )
    # Praw = relu(min(1 - d*c0, 1 + d*c0)) = relu(1 - |d|/vth)
    psiop = mk(
        "ALIF_PSI",
        Spec(
            body=relu(minn(One - Src0 * C0, One + Src0 * C0)),
            reference=lambda in0, in1, s0, s1, imm2: np.maximum(
                0.0, np.minimum(1.0 - np.asarray(in0) * s0, 1.0 + np.asarray(in0) * s0)
            ).astype(np.float32),
        ),
    )
    # q' = (d>0 ? m2*B : m3*B) + n0 - d   (reads PRE-update B)
    qop2 = mk(
        "ALIF_QUPD2",
        Spec(
            body=select(Src0 > Zero, C0 * Src1, C1 * Src1) + C2 - Src0,
            reference=lambda in0, in1, s0, s1, imm2: (
                np.where(np.asarray(in0) > 0, s0 * np.asarray(in1), s1 * np.asarray(in1))
                + imm2 - np.asarray(in0)
            ).astype(np.float32),
        ),
    )
    _OPS_REGISTERED = {"AOP": aop, "QOP": qop, "QOP2": qop2, "PSIOP": psiop}
    return _OPS_REGISTERED


def _const_arrays():
    t = np.arange(T)
    # 0.5-scaled triangular alpha filters (absorb psi's gamma/vth=0.5 factor)
    laT = np.where(t[None, :] >= t[:, None], 0.5 * ALPHA ** (t[None, :] - t[:, None]), 0.0)
    la2T = np.where(t[None, :] - 1 >= t[:, None], 0.5 * ALPHA ** (t[None, :] - 1 - t[:, None]), 0.0)
    kap = np.zeros((128, T * FW), np.float32)
    for tt in range(T):
        kap[:, FW * tt : FW * (tt + 1)] = KAPPA ** (T - 1 - tt)
    ident = np.eye(128, dtype=np.float32)
    return laT.astype(np.float32), la2T.astype(np.float32), kap, ident


def build(nc, debug=False, sim_safe=False):
    """Build the per-core SPMD graph (same on all 8 cores)."""
    import concourse.mybir as mybir
    from concourse.tile import TileContext

    ops = _register_dve_ops()
    AOP, QOP2, PSIOP = ops["AOP"], ops["QOP2"], ops["PSIOP"]
    f32 = mybir.dt.float32
    f32r = mybir.dt.float32r
    Alu = mybir.AluOpType
    Act = mybir.ActivationFunctionType

    def r32(ap):
        return ap.bitcast(f32r)

    acopy_bias = []

    def acopy(dst, src_):
        return nc.scalar.activation(dst, src_, Act.Identity, bias=acopy_bias[0][: src_.shape[0]], scale=1.0)

    laT_np, la2T_np, kap_np, ident_np = _const_arrays()

    # ---- DRAM I/O -------------------------------------------------------
    # bundle_w1: [w_inT(256) | wrec(512)]  (gates the loop)
    # bundle_w3: [woutT(128) | ident(128)] (phase 2 only)
    BM = T * BS + H + HT * H + T * FW
    BW3 = HT * O + 128
    bm_d = nc.dram_tensor("bundle_m", [128, BM], f32, kind="ExternalInput")
    bw3_d = nc.dram_tensor("bundle_w3", [128, BW3], f32, kind="ExternalInput")
    # bundle_x: [x_tbi(512) | laT(48) | la2T(48)]
    bx_d = nc.dram_tensor("bundle_x", [T, BS * I + 2 * T], f32, kind="ExternalInput")
    o_fe_d = nc.dram_tensor("o_fe", [128, BS * HT * (I + H)], f32, kind="ExternalOutput")
    o_ro_d = nc.dram_tensor("o_ro", [O, BS * T], f32, kind="ExternalOutput")
    if debug:
        dbg_z_d = nc.dram_tensor("dbg_z", [128, T * FW], f32, kind="ExternalOutput")
        dbg_d_d = nc.dram_tensor("dbg_d", [128, T * FW], f32, kind="ExternalOutput")
        dbg_w_d = nc.dram_tensor("dbg_w", [128, T * FW], f32, kind="ExternalOutput")


    CHUNK = 8                  # psi/c/kaP computed per 8 finished steps
    CW = CHUNK * FW

    with TileContext(nc) as tc:
        with tc.tile_pool(name="sb", bufs=1) as sb:
            # ---- load inputs to SBUF (bundled DMAs, 2 rings) -----------
            bm = sb.tile([128, BM], f32, tag="bm")
            bw3 = sb.tile([128, BW3], f32, tag="bw3")
            bx = sb.tile([T, BS * I + 2 * T], f32, tag="bx")
            nc.sync.dma_start(bm[:], bm_d.ap())
            nc.scalar.dma_start(bx[:], bx_d.ap())
            nc.scalar.dma_start(bw3[:], bw3_d.ap())
            x_itb = bm[:, : T * BS]
            w_inT = bm[:, T * BS : T * BS + H]
            wrec = bm[:, T * BS + H : T * BS + H + HT * H]
            kap = bm[:, T * BS + H + HT * H :]
            woutT = bw3[:, : HT * O]
            ident = bw3[:, HT * O :]
            x_tbi = bx[:, : BS * I]
            laT = bx[:, BS * I : BS * I + T]
            la2T = bx[:, BS * I + T :]

            # ---- state + history buffers -------------------------------
            q = sb.tile([128, FW], f32, tag="q")
            Bst0 = sb.tile([128, FW], f32, tag="Bst0")
            Bst1 = sb.tile([128, FW], f32, tag="Bst1")
            bpp = [Bst0, Bst1]
            Zbuf = sb.tile([128, T * FW], f32, tag="Zbuf")     # z_t at col FW*t+BS*ht+b
            Dbuf = sb.tile([128, T * FW], f32, tag="Dbuf")     # d_t
            Praw = sb.tile([128, T * FW], f32, tag="Praw")     # 2*psi (in-loop chunks)
            cbuf = sb.tile([128, (T + 1) * FW], f32, tag="cbuf")
            kaP = sb.tile([128, T * FW], f32, tag="kaP")
            b_rho = sb.tile([128, 1], f32, tag="b_rho")
            b_zero = sb.tile([128, 1], f32, tag="b_zero")
            nc.gpsimd.memset(b_zero[:], 0.0)
            acopy_bias.append(b_zero)
            nc.gpsimd.memset(q[:], Q0_)
            nc.gpsimd.memset(Bst0[:], B0N_)
            nc.gpsimd.memset(Bst1[:], B0N_)
            nc.gpsimd.memset(cbuf[:, T * FW :], 0.0)
            nc.gpsimd.memset(b_rho[:], RHO)

            TrCat = sb.tile([T, BS * (I + H)], f32r, tag="TrCat")  # [TrIn_b | TrRec_b]
            x_tbi_r = sb.tile([T, BS * I], f32r, tag="x_tbi_r")
            laT_r = sb.tile([T, T], f32r, tag="laT_r")
            nc.vector.tensor_copy(x_tbi_r[:], x_tbi)
            nc.vector.tensor_copy(laT_r[:], laT)

            with (
                tc.tile_pool(name="ps_i", bufs=1, space="PSUM") as ps_i,
            ):
                # rotating i_t banks: ib[r], layout [ht, t4, b]
                ib = []
                for r in range(NB):
                    bank_t = ps_i.tile([128, HT * TB * BS], f32, tag=f"ib{r}")
                    ib.append(bank_t)

                # ---- the spike recurrence: manually-synced critical section
                s_pe = nc.alloc_semaphore("alif_pe")
                s_dve = nc.alloc_semaphore("alif_dve")
                s_dch = nc.alloc_semaphore("alif_dch")
                s_act = nc.alloc_semaphore("alif_act")
                with tc.tile_critical():
                    for t in range(T):
                        r, t4 = t % NB, t // NB
                        bank = ib[r]

                        def col(ht):
                            return bank[:, ht * TB * BS + t4 * BS : ht * TB * BS + (t4 + 1) * BS]

                        def xp(ht, stop):
                            return nc.tensor.matmul(
                                col(ht),
                                lhsT=w_inT[:, ht * 128 : (ht + 1) * 128],
                                rhs=x_itb[:, BS * t : BS * (t + 1)],
                                start=True, stop=stop,
                            )

                        def rec(ht, kt, stop):
                            return nc.tensor.matmul(
                                col(ht),
                                lhsT=wrec[:, kt * H + ht * 128 : kt * H + (ht + 1) * 128],
                                rhs=Zbuf[:, FW * (t - 1) + BS * kt : FW * (t - 1) + BS * (kt + 1)],
                                start=False, stop=stop,
                            )

                        if t == 0:
                            xp(0, True)
                            xp(1, True).then_inc(s_pe, 1)
                        else:
                            xp(0, False)
                            rec(0, 0, False)._wait_ge(s_dve, t)
                            rec(0, 1, True)
                            xp(1, False)
                            rec(1, 0, False)
                            rec(1, 1, True).then_inc(s_pe, 1)

                        iv = bank[:].rearrange("p (h q b) -> p h q b", h=HT, b=BS)[:, :, t4, :]
                        # z_t = (alpha*q < i_t)  [the only cross-engine handshake]
                        zin = nc.vector.scalar_tensor_tensor(
                            Zbuf[:, FW * t : FW * (t + 1)],
                            in0=q[:], scalar=ALPHA, in1=iv, op0=Alu.mult, op1=Alu.is_lt,
                        )
                        zin._wait_ge(s_pe, t + 1)
                        zin.then_inc(s_dve, 1)
                        # d_t = i_t - alpha*q
                        din = nc.vector.scalar_tensor_tensor(
                            Dbuf[:, FW * t : FW * (t + 1)],
                            in0=q[:], scalar=-ALPHA, in1=iv, op0=Alu.mult, op1=Alu.add,
                        )
                        dsl = Dbuf[:, FW * t : FW * (t + 1)]
                        Bold, Bnew = bpp[t % 2], bpp[1 - t % 2]
                        nc.vector.drain()
                        nc.vector._custom_dve(QOP2, out=q[:], in0=dsl, in1=Bold[:], s0=M2_, s1=M3_, imm2=N0_)
                        nc.vector._custom_dve(AOP, out=Bnew[:], in0=dsl, in1=Bold[:], s0=RHO, s1=K1N_, imm2=K2N_)
                        if sim_safe:
                            nc.vector.drain()

                        # psi/c/kaP chunks on ACT+GPSIMD, overlapped with the loop
                        if t % CHUNK == CHUNK - 1 and t < T - 1:
                            din.then_inc(s_dch, 1)
                            k = t // CHUNK
                            c0 = (t - CHUNK + 1) * FW
                            sl = slice(c0, c0 + CW)
                            nc.scalar.activation(Praw[:, sl], Dbuf[:, sl], Act.Abs)._wait_ge(s_dch, k + 1)
                            nc.scalar.drain()
                            nc.scalar.activation(
                                Praw[:, sl], Praw[:, sl], Act.Relu, bias=1.0, scale=-1.0 / VTH
                            )
                            nc.scalar.drain()
                            nc.scalar.activation(
                                cbuf[:, sl], Praw[:, sl], Act.Identity, bias=b_rho[:], scale=-0.5 * BETA
                            ).then_inc(s_act, 1)
                            nc.gpsimd.tensor_mul(kaP[:, sl], Praw[:, sl], kap[:, sl])._wait_ge(s_act, k + 1)

            # ================= phase 2 (loop PSUM released) =============
            # dummy ACT op: triggers this block's LoadActFuncSet immediately
            nc.scalar.activation(b_rho[:1], b_zero[:1], Act.Identity, bias=b_zero[:1], scale=1.0)
            # last psi/c/kaP chunk on DVE (fast, no cross-engine hops before R)
            lsl = slice((T - CHUNK) * FW, T * FW)
            nc.vector._custom_dve(PSIOP, out=Praw[:, lsl], in0=Dbuf[:, lsl], s0=1.0 / VTH)
            nc.vector.tensor_scalar(cbuf[:, lsl], Praw[:, lsl], -0.5 * BETA, RHO, Alu.mult, Alu.add)
            nc.vector.tensor_mul(kaP[:, lsl], Praw[:, lsl], kap[:, lsl])
            with (
                tc.tile_pool(name="ps_tp", bufs=2, space="PSUM") as ps_tp,
                tc.tile_pool(name="ps_y", bufs=1, space="PSUM") as ps_y,
                tc.tile_pool(name="ps_tr", bufs=2, space="PSUM") as ps_tr,
                tc.tile_pool(name="ps_fe", bufs=3, space="PSUM") as ps_fe,
            ):
                # ---- TrIn = (0.5 La) @ X (PE is warm now) --------------
                trin_ps = ps_tr.tile([T, 512], f32, tag="tr_ps")
                nc.tensor.matmul(trin_ps[:], lhsT=laT_r[:], rhs=x_tbi_r[:], start=True, stop=True)
                for bb2 in range(2):
                    dst = TrCat[:].rearrange("p (b c) -> p b c", b=BS)[:, 2 * bb2 : 2 * bb2 + 2, :I]
                    src_v = trin_ps[:, 256 * bb2 : 256 * (bb2 + 1)].rearrange("p (b c) -> p b c", b=2)
                    nc.vector.tensor_copy(dst, src_v)

                # ---- readout: Y = Z @ w_out.T, kappa-scan over t -------
                y_ps = ps_y.tile([O, T * BS], f32, tag="y")
                z_v = Zbuf[:].rearrange("p (t x) -> p t x", x=FW)
                for kt in range(HT):
                    nc.tensor.matmul(
                        y_ps[:],
                        lhsT=woutT[:, kt * O : (kt + 1) * O],
                        rhs=z_v[:, :, BS * kt : BS * (kt + 1)],
                        start=(kt == 0),
                        stop=(kt == HT - 1),
                    )
                kconst = sb.tile([O, T], f32, tag="kconst")
                nc.gpsimd.memset(kconst[:], KAPPA)
                ro = sb.tile([O, BS * T], f32, tag="ro")
                for b in range(BS):
                    nc.vector.tensor_tensor_scan(
                        ro[:, T * b : T * (b + 1)], kconst[:], y_ps[:, b::BS],
                        0.0, Alu.mult, Alu.add,
                    )

                nc.scalar.activation(la2T_r[:], la2T, Act.Identity, bias=b_zero[:T], scale=1.0)
                nc.vector.tensor_copy(x_tbi_r[:], x_tbi)
                nc.vector.tensor_copy(laT_r[:], laT)
                # ---- Z transposes into (t, h), ht-major ----------------
                Zt = sb.tile([T, BS * H], f32r, tag="Zt")
                for ht in range(HT):
                    tp = ps_tp.tile([T, 512], f32, tag="tp")
                    for b in range(BS):
                        nc.tensor.transpose(tp[:, 128 * b : 128 * (b + 1)], Zbuf[:, BS * ht + b :: FW], ident)
                    cpz = acopy if ht == 0 else nc.vector.tensor_copy
                    cpz(Zt[:, ht * 512 : (ht + 1) * 512], tp[:])

                # ---- R backward scan -> W (per-ht pipelined) -----------
                Rr = sb.tile([128, T * FW], f32, tag="Rr")
                Wr = sb.tile([128, T * FW], f32, tag="Wr")
                Wt = sb.tile([T, BS * H], f32r, tag="Wt")
                wv = Wr[:].rearrange("p (t x) -> p t x", x=FW)
                rv = Rr[:].rearrange("p (t x) -> p t x", x=FW)
                pv = Praw[:].rearrange("p (t x) -> p t x", x=FW)
                kv = kap.rearrange("p (t x) -> p t x", x=FW)
                for ht in range(HT):
                    for b in range(BS):
                        j = BS * ht + b
                        nc.vector.tensor_tensor_scan(
                            Rr[:, FW * (T - 1) + j :: -FW],
                            cbuf[:, FW * T + j : j : -FW],
                            kaP[:, FW * (T - 1) + j :: -FW],
                            0.0, Alu.mult, Alu.add,
                        )
                    hsl = slice(BS * ht, BS * (ht + 1))
                    nc.vector.scalar_tensor_tensor(
                        wv[:, :, hsl], in0=rv[:, :, hsl], scalar=-0.5 * BETA,
                        in1=kv[:, :, hsl], op0=Alu.mult, op1=Alu.add,
                    )
                    nc.vector.tensor_tensor(wv[:, :, hsl], pv[:, :, hsl], wv[:, :, hsl], Alu.mult)
                    tp = ps_tp.tile([T, 512], f32, tag="tp")
                    for b in range(BS):
                        nc.tensor.transpose(tp[:, 128 * b : 128 * (b + 1)], Wr[:, BS * ht + b :: FW], ident)
                    for b in range(BS):
                        cpw = acopy if (b + ht) % 2 == 0 else nc.vector.tensor_copy
                        cpw(
                            Wt[:, ht * 512 + b * 128 : ht * 512 + (b + 1) * 128],
                            tp[:, b * 128 : (b + 1) * 128],
                        )

                # ---- TrRec = (0.5 La'') @ Z into TrCat -----------------
                la2T_r = sb.tile([T, T], f32r, tag="la2T_r")
                nc.vector.tensor_copy(la2T_r[:], la2T)
                for ht in range(HT):
                    trr_ps = ps_tr.tile([T, 512], f32, tag="tr_ps")
                    nc.tensor.matmul(
                        trr_ps[:], lhsT=la2T_r[:], rhs=Zt[:, 512 * ht : 512 * (ht + 1)],
                        start=True, stop=True,
                    )
                    for j in range(2):
                        dst = TrCat[:].rearrange("p (b c) -> p b c", b=BS)[
                            :, 2 * j : 2 * j + 2, I + ht * 128 : I + (ht + 1) * 128
                        ]
                        src_v = trr_ps[:, 256 * j : 256 * (j + 1)].rearrange(
                            "p (b c) -> p b c", b=2
                        )
                        cp = acopy if (ht + j) % 2 == 0 else nc.vector.tensor_copy
                        cp(dst, src_v)

                # ---- fe = W^T @ [TrIn | TrRec] per (ht, b) -------------
                fe_sb = sb.tile([128, BS * HT * (I + H)], f32, tag="fe_sb")
                for ht in range(HT):
                    for b in range(BS):
                        fe_ps = ps_fe.tile([128, I + H], f32, tag="fe_ps")
                        nc.tensor.matmul(
                            fe_ps[:],
                            lhsT=Wt[:, ht * 512 + b * 128 : ht * 512 + (b + 1) * 128],
                            rhs=TrCat[:, b * (I + H) : (b + 1) * (I + H)],
                            start=True, stop=True,
                        )
                        blk = ht * BS + b
                        dst = fe_sb[:, blk * (I + H) : (blk + 1) * (I + H)]
                        cp = acopy if (b + ht) % 2 == 0 else nc.vector.tensor_copy
                        cp(dst, fe_ps[:])
                        if blk % 2 == 1:
                            lo = (blk - 1) * (I + H)
                            hi = (blk + 1) * (I + H)
                            nc.sync.dma_start(o_fe_d.ap()[:, lo:hi], fe_sb[:, lo:hi])

                nc.scalar.dma_start(o_ro_d.ap(), ro[:])

                if debug:
                    nc.sync.dma_start(dbg_z_d.ap(), Zbuf[:])
                    nc.sync.dma_start(dbg_d_d.ap(), Dbuf[:])
                    nc.sync.dma_start(dbg_w_d.ap(), Wr[:])
    return nc


def prepare_in_maps(x_seq, w_in, w_rec, w_out):
    x_seq = np.ascontiguousarray(x_seq, np.float32)
    w_inT = np.ascontiguousarray(w_in.T, np.float32)
    w_recT2 = np.ascontiguousarray(
        w_rec.T.reshape(HT, 128, H).transpose(1, 0, 2).reshape(128, HT * H), np.float32
    )
    w_outT2 = np.ascontiguousarray(
        w_out.T.reshape(HT, 128, O).transpose(1, 0, 2).reshape(128, HT * O), np.float32
    )
    laT_np, la2T_np, kap_np, ident_np = _const_arrays()
    bundle_w3 = np.ascontiguousarray(np.concatenate([w_outT2, ident_np], axis=1))
    in_maps = []
    for c in range(NCORES):
        xc = x_seq[c * BS : (c + 1) * BS]  # (4, 48, 128)
        x_tbi = xc.transpose(1, 0, 2).reshape(T, BS * I)
        bundle_x = np.ascontiguousarray(
            np.concatenate([x_tbi, laT_np, la2T_np], axis=1)
        )
        in_maps.append(
            {
                "bundle_m": np.ascontiguousarray(
                    np.concatenate(
                        [xc.transpose(2, 1, 0).reshape(I, T * BS), w_inT, w_recT2, kap_np],
                        axis=1,
                    )
                ),
                "bundle_w3": bundle_w3,
                "bundle_x": bundle_x,
            }
        )
    return in_maps


def assemble_outputs(results):
    readout = np.empty((B, T, O), np.float32)
    fe_in = np.empty((B, H, I), np.float32)
    fe_rec = np.empty((B, H, H), np.float32)
    for c in range(NCORES):
        o_fe = results[c]["o_fe"]  # (128, BS*HT*(I+H))
        o_ro = results[c]["o_ro"]  # (O, BS*T)
        fe = o_fe.reshape(128, HT, BS, I + H)
        for b in range(BS):
            g = c * BS + b
            readout[g] = o_ro[:, b * T : (b + 1) * T].T
            for ht in range(HT):
                fe_in[g, ht * 128 : (ht + 1) * 128, :] = fe[:, ht, b, :I]
                fe_rec[g, ht * 128 : (ht + 1) * 128, :] = fe[:, ht, b, I:]
    return readout, (fe_in, fe_rec)


_COMPILED = {}


def _get_compiled():
    if "nc" not in _COMPILED:
        import concourse.bacc as bacc

        nc = bacc.Bacc("TRN2", target_bir_lowering=False, debug=False, num_devices=NCORES)
        build(nc, debug=False)
        nc.compile()
        _COMPILED["nc"] = nc
    return _COMPILED["nc"]


def kernel(x_seq, w_in, w_rec, w_out):
    from concourse import bass_utils

    nc = _get_compiled()
    in_maps = prepare_in_maps(x_seq, w_in, w_rec, w_out)
    res = bass_utils.run_bass_kernel_spmd(nc, in_maps, core_ids=list(range(NCORES)))
    return assemble_outputs(res.results)


# revision 58
# speedup vs baseline: 1.0027x; 1.0027x over previous
"""ALIF spiking-network forward + eligibility traces on 8 Trainium2 NeuronCores.

Data-parallel: batch 32 sharded 4 samples/core. The only sequential part is the
(B,H) spike recurrence (48 steps); the heavy per-sample eligibility-trace
matrices (eps/fe, H x I and H x H) are collapsed algebraically into per-(h,t)
scalars + a few small matmuls:

  eps_t = (rho - beta*psi_t) eps_{t-1} + psi_t (x) tr_t   (rank-1, row decay)
  fe_T  = sum_s W_s (x) tr_s,  W_s = psi_s (kap_s - beta R_s),
  R_s   = kap_s psi_s + c_{s+1} R_{s+1}                   (backward linear scan)
  tr    = La @ X  /  La'' @ Z                             (triangular matmul)

so  fe_in = W^T @ (La @ X),  fe_rec = W^T @ (La'' @ Z).
"""

import numpy as np

# ---------------------------------------------------------------- constants
ALPHA = float(np.exp(-1.0 / 20.0))
RHO = float(np.exp(-1.0 / 2000.0))
KAPPA = float(np.exp(-1.0 / 20.0))
BETA = 0.07
VTH = 0.6
GAMMA = 0.3

B, T, I, H, O = 32, 48, 128, 256, 64
NCORES = 8
BS = B // NCORES          # 4 samples per core
HT = H // 128             # 2 h-tiles
FW = HT * BS              # 8 = free width of loop state tiles
NB = 4                    # i_t PSUM bank rotation depth
TB = T // NB              # 12 time-cols per bank

C1_ = VTH * (1.0 - RHO)
W_ = 1.0 / ALPHA - 1.0 / RHO
U_ = C1_ / RHO
K1_ = W_ * (BETA + C1_) + U_ * (1.0 - RHO)
K2_ = W_ * C1_ + U_ * (1.0 - RHO)
M1_ = 1.0 / (ALPHA * W_)
N1_ = (VTH - U_ / W_) / ALPHA
Q0_ = VTH / ALPHA
B0_ = W_ * VTH + U_
# pre-update-B formulation (QOp reads B before AOp updates it)
UT_ = (BETA + VTH) / ALPHA
B0N_ = VTH + UT_
K1N_ = BETA + C1_ + UT_ * (1.0 - RHO)
K2N_ = C1_ + UT_ * (1.0 - RHO)
M2_ = RHO / ALPHA
M3_ = RHO / ALPHA - 1.0
N0_ = C1_ / ALPHA - (RHO / ALPHA - 1.0) * UT_

_OPS_REGISTERED = {}


def _register_dve_ops():
    """Two fused DVE ops for the loop state updates (idempotent)."""
    global _OPS_REGISTERED
    if _OPS_REGISTERED:
        return _OPS_REGISTERED
    import concourse.dve_ops as dve_ops
    from concourse.dve_ops import OPS, DveOp
    from concourse.dve_spec import Spec, Src0, Src1, C0, C1, C2, select, Zero, One, lower, relu, minn
    from concourse.dve_uop import DveOpSpec

    def mk(name, spec):
        for o in OPS:
            if o.name == name:
                return o
        from concourse.dve_ops import has_src1
        shas = {}
        for ver in ("v3", "v4"):
            s = DveOpSpec(name=name, opcode=0, uops=lower(spec, ver=ver), rd1_en=has_src1(spec))
            shas[ver] = s.sha(ver)
        op = DveOp(name, spec, subdim=False, uops_sha=shas)
        OPS.append(op)
        dve_ops.CUSTOM_DVE_SPECS[name] = spec
        dve_ops._SUB_OPCODE_FOR_NAME[name] = dve_ops._CUSTOM_DVE_ROW_BASE + len(OPS) - 1
        assert dve_ops._SUB_OPCODE_FOR_NAME[name] < 0x20
        return op

    # B' = rho*B + (d>0 ? k1 : k2)      (B = affine-transformed threshold state)
    aop = mk(
        "ALIF_BUPD",
        Spec(
            body=Src1 * C0 + select(Src0 > Zero, C1, C2),
            reference=lambda in0, in1, s0, s1, imm2: (
                np.asarray(in1) * s0 + np.where(np.asarray(in0) > 0, s1, imm2)
            ).astype(np.float32),
        ),
    )
    # q' = (d>0 ? m1*B' + n1 : B') - d   (q: next-step threshold, d_{t+1}=i-alpha*q)
    qop = mk(
        "ALIF_QUPD",
        Spec(
            body=select(Src0 > Zero, C0 * Src1 + C1, Src1) - Src0,
            reference=lambda in0, in1, s0, s1, imm2: (
                np.where(np.asarray(in0) > 0, s0 * np.asarray(in1) + s1, np.asarray(in1))
                - np.asarray(in0)
            ).astype(np.float32),
        ),
    )
    # Praw = relu(min(1 - d*c0, 1 + d*c0)) = relu(1 - |d|/vth)
    psiop = mk(
        "ALIF_PSI",
        Spec(
            body=relu(minn(One - Src0 * C0, One + Src0 * C0)),
            reference=lambda in0, in1, s0, s1, imm2: np.maximum(
                0.0, np.minimum(1.0 - np.asarray(in0) * s0, 1.0 + np.asarray(in0) * s0)
            ).astype(np.float32),
        ),
    )
    # q' = (d>0 ? m2*B : m3*B) + n0 - d   (reads PRE-update B)
    qop2 = mk(
        "ALIF_QUPD2",
        Spec(
            body=select(Src0 > Zero, C0 * Src1, C1 * Src1) + C2 - Src0,
            reference=lambda in0, in1, s0, s1, imm2: (
                np.where(np.asarray(in0) > 0, s0 * np.asarray(in1), s1 * np.asarray(in1))
                + imm2 - np.asarray(in0)
            ).astype(np.float32),
        ),
    )
    _OPS_REGISTERED = {"AOP": aop, "QOP": qop, "QOP2": qop2, "PSIOP": psiop}
    return _OPS_REGISTERED


def _const_arrays():
    t = np.arange(T)
    # 0.5-scaled triangular alpha filters (absorb psi's gamma/vth=0.5 factor)
    laT = np.where(t[None, :] >= t[:, None], 0.5 * ALPHA ** (t[None, :] - t[:, None]), 0.0)
    la2T = np.where(t[None, :] - 1 >= t[:, None], 0.5 * ALPHA ** (t[None, :] - 1 - t[:, None]), 0.0)
    kap = np.zeros((128, T * FW), np.float32)
    for tt in range(T):
        kap[:, FW * tt : FW * (tt + 1)] = KAPPA ** (T - 1 - tt)
    ident = np.eye(128, dtype=np.float32)
    return laT.astype(np.float32), la2T.astype(np.float32), kap, ident


def build(nc, debug=False, sim_safe=False):
    """Build the per-core SPMD graph (same on all 8 cores)."""
    import concourse.mybir as mybir
    from concourse.tile import TileContext

    ops = _register_dve_ops()
    AOP, QOP2, PSIOP = ops["AOP"], ops["QOP2"], ops["PSIOP"]
    f32 = mybir.dt.float32
    f32r = mybir.dt.float32r
    Alu = mybir.AluOpType
    Act = mybir.ActivationFunctionType

    def r32(ap):
        return ap.bitcast(f32r)

    acopy_bias = []

    def acopy(dst, src_):
        return nc.scalar.activation(dst, src_, Act.Identity, bias=acopy_bias[0][: src_.shape[0]], scale=1.0)

    laT_np, la2T_np, kap_np, ident_np = _const_arrays()

    # ---- DRAM I/O -------------------------------------------------------
    # bundle_w1: [w_inT(256) | wrec(512)]  (gates the loop)
    # bundle_w3: [woutT(128) | ident(128)] (phase 2 only)
    BM = T * BS + H + HT * H + T * FW
    BW3 = HT * O + 128
    bm_d = nc.dram_tensor("bundle_m", [128, BM], f32, kind="ExternalInput")
    bw3_d = nc.dram_tensor("bundle_w3", [128, BW3], f32, kind="ExternalInput")
    # bundle_x: [x_tbi(512) | laT(48) | la2T(48)]
    bx_d = nc.dram_tensor("bundle_x", [T, BS * I + 2 * T], f32, kind="ExternalInput")
    o_fe_d = nc.dram_tensor("o_fe", [128, BS * HT * (I + H)], f32, kind="ExternalOutput")
    o_ro_d = nc.dram_tensor("o_ro", [O, BS * T], f32, kind="ExternalOutput")
    if debug:
        dbg_z_d = nc.dram_tensor("dbg_z", [128, T * FW], f32, kind="ExternalOutput")
        dbg_d_d = nc.dram_tensor("dbg_d", [128, T * FW], f32, kind="ExternalOutput")
        dbg_w_d = nc.dram_tensor("dbg_w", [128, T * FW], f32, kind="ExternalOutput")


    CHUNK = 8                  # psi/c/kaP computed per 8 finished steps
    CW = CHUNK * FW

    with TileContext(nc) as tc:
        with tc.tile_pool(name="sb", bufs=1) as sb:
            # ---- load inputs to SBUF (bundled DMAs, 2 rings) -----------
            bm = sb.tile([128, BM], f32, tag="bm")
            bw3 = sb.tile([128, BW3], f32, tag="bw3")
            bx = sb.tile([T, BS * I + 2 * T], f32, tag="bx")
            nc.sync.dma_start(bm[:], bm_d.ap())
            nc.scalar.dma_start(bx[:], bx_d.ap())
            nc.scalar.dma_start(bw3[:], bw3_d.ap())
            x_itb = bm[:, : T * BS]
            w_inT = bm[:, T * BS : T * BS + H]
            wrec = bm[:, T * BS + H : T * BS + H + HT * H]
            kap = bm[:, T * BS + H + HT * H :]
            woutT = bw3[:, : HT * O]
            ident = bw3[:, HT * O :]
            x_tbi = bx[:, : BS * I]
            laT = bx[:, BS * I : BS * I + T]
            la2T = bx[:, BS * I + T :]

            # ---- state + history buffers -------------------------------
            q = sb.tile([128, FW], f32, tag="q")
            Bst0 = sb.tile([128, FW], f32, tag="Bst0")
            Bst1 = sb.tile([128, FW], f32, tag="Bst1")
            bpp = [Bst0, Bst1]
            Zbuf = sb.tile([128, T * FW], f32, tag="Zbuf")     # z_t at col FW*t+BS*ht+b
            Dbuf = sb.tile([128, T * FW], f32, tag="Dbuf")     # d_t
            Praw = sb.tile([128, T * FW], f32, tag="Praw")     # 2*psi (in-loop chunks)
            cbuf = sb.tile([128, (T + 1) * FW], f32, tag="cbuf")
            kaP = sb.tile([128, T * FW], f32, tag="kaP")
            b_rho = sb.tile([128, 1], f32, tag="b_rho")
            b_zero = sb.tile([128, 1], f32, tag="b_zero")
            nc.gpsimd.memset(b_zero[:], 0.0)
            acopy_bias.append(b_zero)
            nc.gpsimd.memset(q[:], Q0_)
            nc.gpsimd.memset(Bst0[:], B0N_)
            nc.gpsimd.memset(Bst1[:], B0N_)
            nc.gpsimd.memset(cbuf[:, T * FW :], 0.0)
            nc.gpsimd.memset(b_rho[:], RHO)

            TrCat = sb.tile([T, BS * (I + H)], f32r, tag="TrCat")  # [TrIn_b | TrRec_b]
            x_tbi_r = sb.tile([T, BS * I], f32r, tag="x_tbi_r")
            laT_r = sb.tile([T, T], f32r, tag="laT_r")
            nc.vector.tensor_copy(x_tbi_r[:], x_tbi)
            nc.vector.tensor_copy(laT_r[:], laT)

            with (
                tc.tile_pool(name="ps_i", bufs=1, space="PSUM") as ps_i,
            ):
                # rotating i_t banks: ib[r], layout [ht, t4, b]
                ib = []
                for r in range(NB):
                    bank_t = ps_i.tile([128, HT * TB * BS], f32, tag=f"ib{r}")
                    ib.append(bank_t)

                # ---- the spike recurrence: manually-synced critical section
                s_pe = nc.alloc_semaphore("alif_pe")
                s_dve = nc.alloc_semaphore("alif_dve")
                s_dch = nc.alloc_semaphore("alif_dch")
                s_act = nc.alloc_semaphore("alif_act")
                with tc.tile_critical():
                    for t in range(T):
                        r, t4 = t % NB, t // NB
                        bank = ib[r]

                        def col(ht):
                            return bank[:, ht * TB * BS + t4 * BS : ht * TB * BS + (t4 + 1) * BS]

                        def xp(ht, stop):
                            return nc.tensor.matmul(
                                col(ht),
                                lhsT=w_inT[:, ht * 128 : (ht + 1) * 128],
                                rhs=x_itb[:, BS * t : BS * (t + 1)],
                                start=True, stop=stop,
                            )

                        def rec(ht, kt, stop):
                            return nc.tensor.matmul(
                                col(ht),
                                lhsT=wrec[:, kt * H + ht * 128 : kt * H + (ht + 1) * 128],
                                rhs=Zbuf[:, FW * (t - 1) + BS * kt : FW * (t - 1) + BS * (kt + 1)],
                                start=False, stop=stop,
                            )

                        if t == 0:
                            xp(0, True)
                            xp(1, True).then_inc(s_pe, 1)
                        else:
                            xp(0, False)
                            rec(0, 0, False)._wait_ge(s_dve, t)
                            rec(0, 1, True)
                            xp(1, False)
                            rec(1, 0, False)
                            rec(1, 1, True).then_inc(s_pe, 1)

                        iv = bank[:].rearrange("p (h q b) -> p h q b", h=HT, b=BS)[:, :, t4, :]
                        # z_t = (alpha*q < i_t)  [the only cross-engine handshake]
                        zin = nc.vector.scalar_tensor_tensor(
                            Zbuf[:, FW * t : FW * (t + 1)],
                            in0=q[:], scalar=ALPHA, in1=iv, op0=Alu.mult, op1=Alu.is_lt,
                        )
                        zin._wait_ge(s_pe, t + 1)
                        zin.then_inc(s_dve, 1)
                        # d_t = i_t - alpha*q
                        din = nc.vector.scalar_tensor_tensor(
                            Dbuf[:, FW * t : FW * (t + 1)],
                            in0=q[:], scalar=-ALPHA, in1=iv, op0=Alu.mult, op1=Alu.add,
                        )
                        dsl = Dbuf[:, FW * t : FW * (t + 1)]
                        Bold, Bnew = bpp[t % 2], bpp[1 - t % 2]
                        nc.vector.drain()
                        nc.vector._custom_dve(QOP2, out=q[:], in0=dsl, in1=Bold[:], s0=M2_, s1=M3_, imm2=N0_)
                        nc.vector._custom_dve(AOP, out=Bnew[:], in0=dsl, in1=Bold[:], s0=RHO, s1=K1N_, imm2=K2N_)
                        if sim_safe:
                            nc.vector.drain()

                        # psi/c/kaP chunks on ACT+GPSIMD, overlapped with the loop
                        if t % CHUNK == CHUNK - 1 and t < T - 1:
                            din.then_inc(s_dch, 1)
                            k = t // CHUNK
                            c0 = (t - CHUNK + 1) * FW
                            sl = slice(c0, c0 + CW)
                            nc.scalar.activation(Praw[:, sl], Dbuf[:, sl], Act.Abs)._wait_ge(s_dch, k + 1)
                            nc.scalar.drain()
                            nc.scalar.activation(
                                Praw[:, sl], Praw[:, sl], Act.Relu, bias=1.0, scale=-1.0 / VTH
                            )
                            nc.scalar.drain()
                            nc.scalar.activation(
                                cbuf[:, sl], Praw[:, sl], Act.Identity, bias=b_rho[:], scale=-0.5 * BETA
                            ).then_inc(s_act, 1)
                            nc.gpsimd.tensor_mul(kaP[:, sl], Praw[:, sl], kap[:, sl])._wait_ge(s_act, k + 1)

            # ================= phase 2 (loop PSUM released) =============
            # dummy ACT op: triggers this block's LoadActFuncSet immediately
            nc.scalar.activation(b_rho[:1], b_zero[:1], Act.Identity, bias=b_zero[:1], scale=1.0)
            # last psi/c/kaP chunk on DVE (fast, no cross-engine hops before R)
            lsl = slice((T - CHUNK) * FW, T * FW)
            nc.vector._custom_dve(PSIOP, out=Praw[:, lsl], in0=Dbuf[:, lsl], s0=1.0 / VTH)
            nc.vector.tensor_scalar(cbuf[:, lsl], Praw[:, lsl], -0.5 * BETA, RHO, Alu.mult, Alu.add)
            nc.vector.tensor_mul(kaP[:, lsl], Praw[:, lsl], kap[:, lsl])
            with (
                tc.tile_pool(name="ps_tp", bufs=2, space="PSUM") as ps_tp,
                tc.tile_pool(name="ps_y", bufs=1, space="PSUM") as ps_y,
                tc.tile_pool(name="ps_tr", bufs=2, space="PSUM") as ps_tr,
                tc.tile_pool(name="ps_fe", bufs=3, space="PSUM") as ps_fe,
            ):
                # ---- TrIn = (0.5 La) @ X (PE is warm now) --------------
                trin_ps = ps_tr.tile([T, 512], f32, tag="tr_ps")
                nc.tensor.matmul(trin_ps[:], lhsT=laT_r[:], rhs=x_tbi_r[:], start=True, stop=True)
                for bb2 in range(2):
                    dst = TrCat[:].rearrange("p (b c) -> p b c", b=BS)[:, 2 * bb2 : 2 * bb2 + 2, :I]
                    src_v = trin_ps[:, 256 * bb2 : 256 * (bb2 + 1)].rearrange("p (b c) -> p b c", b=2)
                    nc.vector.tensor_copy(dst, src_v)

                # ---- fe = W^T @ [TrIn | TrRec] per (ht, b) -------------
                fe_sb = sb.tile([128, BS * HT * (I + H)], f32, tag="fe_sb")
                for ht in range(HT):
                    for b in range(BS):
                        fe_ps = ps_fe.tile([128, I + H], f32, tag="fe_ps")
                        nc.tensor.matmul(
                            fe_ps[:],
                            lhsT=Wt[:, ht * 512 + b * 128 : ht * 512 + (b + 1) * 128],
                            rhs=TrCat[:, b * (I + H) : (b + 1) * (I + H)],
                            start=True, stop=True,
                        )
                        blk = ht * BS + b
                        dst = fe_sb[:, blk * (I + H) : (blk + 1) * (I + H)]
                        cp = acopy if (b + ht) % 2 == 0 else nc.vector.tensor_copy
                        cp(dst, fe_ps[:])
                        if blk % 2 == 1:
                            lo = (blk - 1) * (I + H)
                            hi = (blk + 1) * (I + H)
                            nc.sync.dma_start(o_fe_d.ap()[:, lo:hi], fe_sb[:, lo:hi])

                # ---- readout: Y = Z @ w_out.T, kappa-scan over t -------
                y_ps = ps_y.tile([O, T * BS], f32, tag="y")
                z_v = Zbuf[:].rearrange("p (t x) -> p t x", x=FW)
                for kt in range(HT):
                    nc.tensor.matmul(
                        y_ps[:],
                        lhsT=woutT[:, kt * O : (kt + 1) * O],
                        rhs=z_v[:, :, BS * kt : BS * (kt + 1)],
                        start=(kt == 0),
                        stop=(kt == HT - 1),
                    )
                kconst = sb.tile([O, T], f32, tag="kconst")
                nc.gpsimd.memset(kconst[:], KAPPA)
                ro = sb.tile([O, BS * T], f32, tag="ro")
                for b in range(BS):
                    nc.vector.tensor_tensor_scan(
                        ro[:, T * b : T * (b + 1)], kconst[:], y_ps[:, b::BS],
                        0.0, Alu.mult, Alu.add,
                    )

                nc.scalar.activation(la2T_r[:], la2T, Act.Identity, bias=b_zero[:T], scale=1.0)
                nc.vector.tensor_copy(x_tbi_r[:], x_tbi)
                nc.vector.tensor_copy(laT_r[:], laT)
                # ---- Z transposes into (t, h), ht-major ----------------
                Zt = sb.tile([T, BS * H], f32r, tag="Zt")
                for ht in range(HT):
                    tp = ps_tp.tile([T, 512], f32, tag="tp")
                    for b in range(BS):
                        nc.tensor.transpose(tp[:, 128 * b : 128 * (b + 1)], Zbuf[:, BS * ht + b :: FW], ident)
                    cpz = acopy if ht == 0 else nc.vector.tensor_copy
                    cpz(Zt[:, ht * 512 : (ht + 1) * 512], tp[:])

                # ---- R backward scan -> W (per-ht pipelined) -----------
                Rr = sb.tile([128, T * FW], f32, tag="Rr")
                Wr = sb.tile([128, T * FW], f32, tag="Wr")
                Wt = sb.tile([T, BS * H], f32r, tag="Wt")
                wv = Wr[:].rearrange("p (t x) -> p t x", x=FW)
                rv = Rr[:].rearrange("p (t x) -> p t x", x=FW)
                pv = Praw[:].rearrange("p (t x) -> p t x", x=FW)
                kv = kap.rearrange("p (t x) -> p t x", x=FW)
                for ht in range(HT):
                    for b in range(BS):
                        j = BS * ht + b
                        nc.vector.tensor_tensor_scan(
                            Rr[:, FW * (T - 1) + j :: -FW],
                            cbuf[:, FW * T + j : j : -FW],
                            kaP[:, FW * (T - 1) + j :: -FW],
                            0.0, Alu.mult, Alu.add,
                        )
                    hsl = slice(BS * ht, BS * (ht + 1))
                    nc.vector.scalar_tensor_tensor(
                        wv[:, :, hsl], in0=rv[:, :, hsl], scalar=-0.5 * BETA,
                        in1=kv[:, :, hsl], op0=Alu.mult, op1=Alu.add,
                    )
                    nc.vector.tensor_tensor(wv[:, :, hsl], pv[:, :, hsl], wv[:, :, hsl], Alu.mult)
                    tp = ps_tp.tile([T, 512], f32, tag="tp")
                    for b in range(BS):
                        nc.tensor.transpose(tp[:, 128 * b : 128 * (b + 1)], Wr[:, BS * ht + b :: FW], ident)
                    for b in range(BS):
                        cpw = acopy if (b + ht) % 2 == 0 else nc.vector.tensor_copy
                        cpw(
                            Wt[:, ht * 512 + b * 128 : ht * 512 + (b + 1) * 128],
                            tp[:, b * 128 : (b + 1) * 128],
                        )

                # ---- TrRec = (0.5 La'') @ Z into TrCat -----------------
                la2T_r = sb.tile([T, T], f32r, tag="la2T_r")
                nc.vector.tensor_copy(la2T_r[:], la2T)
                for ht in range(HT):
                    trr_ps = ps_tr.tile([T, 512], f32, tag="tr_ps")
                    nc.tensor.matmul(
                        trr_ps[:], lhsT=la2T_r[:], rhs=Zt[:, 512 * ht : 512 * (ht + 1)],
                        start=True, stop=True,
                    )
                    for j in range(2):
                        dst = TrCat[:].rearrange("p (b c) -> p b c", b=BS)[
                            :, 2 * j : 2 * j + 2, I + ht * 128 : I + (ht + 1) * 128
                        ]
                        src_v = trr_ps[:, 256 * j : 256 * (j + 1)].rearrange(
                            "p (b c) -> p b c", b=2
                        )
                        cp = acopy if (ht + j) % 2 == 0 else nc.vector.tensor_copy
                        cp(dst, src_v)

                nc.scalar.dma_start(o_ro_d.ap(), ro[:])

                if debug:
                    nc.sync.dma_start(dbg_z_d.ap(), Zbuf[:])
                    nc.sync.dma_start(dbg_d_d.ap(), Dbuf[:])
                    nc.sync.dma_start(dbg_w_d.ap(), Wr[:])
    return nc


def prepare_in_maps(x_seq, w_in, w_rec, w_out):
    x_seq = np.ascontiguousarray(x_seq, np.float32)
    w_inT = np.ascontiguousarray(w_in.T, np.float32)
    w_recT2 = np.ascontiguousarray(
        w_rec.T.reshape(HT, 128, H).transpose(1, 0, 2).reshape(128, HT * H), np.float32
    )
    w_outT2 = np.ascontiguousarray(
        w_out.T.reshape(HT, 128, O).transpose(1, 0, 2).reshape(128, HT * O), np.float32
    )
    laT_np, la2T_np, kap_np, ident_np = _const_arrays()
    bundle_w3 = np.ascontiguousarray(np.concatenate([w_outT2, ident_np], axis=1))
    in_maps = []
    for c in range(NCORES):
        xc = x_seq[c * BS : (c + 1) * BS]  # (4, 48, 128)
        x_tbi = xc.transpose(1, 0, 2).reshape(T, BS * I)
        bundle_x = np.ascontiguousarray(
            np.concatenate([x_tbi, laT_np, la2T_np], axis=1)
        )
        in_maps.append(
            {
                "bundle_m": np.ascontiguousarray(
                    np.concatenate(
                        [xc.transpose(2, 1, 0).reshape(I, T * BS), w_inT, w_recT2, kap_np],
                        axis=1,
                    )
                ),
                "bundle_w3": bundle_w3,
                "bundle_x": bundle_x,
            }
        )
    return in_maps


def assemble_outputs(results):
    readout = np.empty((B, T, O), np.float32)
    fe_in = np.empty((B, H, I), np.float32)
    fe_rec = np.empty((B, H, H), np.float32)
    for c in range(NCORES):
        o_fe = results[c]["o_fe"]  # (128, BS*HT*(I+H))
        o_ro = results[c]["o_ro"]  # (O, BS*T)
        fe = o_fe.reshape(128, HT, BS, I + H)
        for b in range(BS):
            g = c * BS + b
            readout[g] = o_ro[:, b * T : (b + 1) * T].T
            for ht in range(HT):
                fe_in[g, ht * 128 : (ht + 1) * 128, :] = fe[:, ht, b, :I]
                fe_rec[g, ht * 128 : (ht + 1) * 128, :] = fe[:, ht, b, I:]
    return readout, (fe_in, fe_rec)


_COMPILED = {}


def _get_compiled():
    if "nc" not in _COMPILED:
        import concourse.bacc as bacc

        nc = bacc.Bacc("TRN2", target_bir_lowering=False, debug=False, num_devices=NCORES)
        build(nc, debug=False)
        nc.compile()
        _COMPILED["nc"] = nc
    return _COMPILED["nc"]


def kernel(x_seq, w_in, w_rec, w_out):
    from concourse import bass_utils

    nc = _get_compiled()
    in_maps = prepare_in_maps(x_seq, w_in, w_rec, w_out)
    res = bass_utils.run_bass_kernel_spmd(nc, in_maps, core_ids=list(range(NCORES)))
    return assemble_outputs(res.results)
